# revision 9
# baseline (speedup 1.0000x reference)
"""Trainium2 Bass kernel for: ConvTranspose2d(128->256, k=4, s=2, p=1)
-> MaxPool2d(2,2) -> Hardtanh -> spatial mean -> Tanh.

Key algebraic restructuring: the stride-2 transposed conv decomposes into 4
polyphase 2x2 convolutions, and the outputs of the 4 phases at pooled
position (i, j) are exactly the 4 elements of the 2x2 maxpool window at
(i, j).  So the 128x128 conv-transpose output is never materialized:

    pooled[b, co, i, j] = max_phase  conv2x2_phase(x)[b, co, i, j] + bias

Everything stays at 64x64 resolution.  Each phase conv is 4 accumulating
K=128 matmuls on the PE array (Cin on partitions, Cout in two 128-halves).
The taps' spatial shifts are realized as AP offsets into a zero-padded
66x66 "canvas" copy of the image in SBUF; the moving operand is a
[8 rows x 64 cols] strided view so only valid output columns are computed
and each 8-row chunk exactly fills one PSUM bank (512 fp32).

Sharding: data-parallel over batch, 8 images per core on 8 cores.
Weights (tiny) replicated.  Matmuls in bf16 (fp32 matmul is 2x slower on
the PE; bf16 keeps ~2e-4 relative error here), accumulation in fp32 PSUM,
phase-max/clip tree in bf16 on DVE, mean+tanh in fp32 on ACT/DVE.
"""

from contextlib import ExitStack

import ml_dtypes
import numpy as np

import concourse.bacc as bacc
import concourse.bass as bass
import concourse.mybir as mybir
import concourse.tile as tile
from concourse.bass_utils import run_bass_kernel_spmd

# Problem dims (hardcoded per contract)
B, CIN, COUT, H, W = 64, 128, 256, 64, 64
NCORES = 8
BPC = B // NCORES  # images per core

WP = 66  # padded row width (1 + 64 + 1)
NROW = 66  # padded rows (1 + 64 + 1)
CVTOT = WP * NROW  # 4356

# Output rows r=1..64 of the canvas grid, 8 chunks x 8 rows; each chunk's
# [8 x 64] valid-column block exactly fills one PSUM bank.
NCHUNK = 8
CHUNK_ROWS = [8] * NCHUNK
CHUNK_R0 = [1 + 8 * i for i in range(NCHUNK)]
GROUPS = [[0, 1, 2, 3], [4, 5, 6, 7]]

F32 = mybir.dt.float32
BF16 = mybir.dt.bfloat16


def _tap(ph: int, a: int):
    """For phase parity ph (0=even output coord, 1=odd) and tap index a,
    return (input shift, kernel index) in one dimension.

    ConvTranspose2d(stride=2, pad=1): out[2q+r] = sum over taps of
    x[q+di] * w[k].  r=0: (di,k) in {(0,1), (-1,3)}; r=1: {(1,0), (0,2)}.
    """
    if ph == 0:
        return (0, 1) if a == 0 else (-1, 3)
    return (1, 0) if a == 0 else (0, 2)


def _wcol(half: int, p: int, t: int) -> int:
    return ((half * 4 + p) * 4 + t) * 128


def build_nc(n_imgs: int = BPC, n_halves: int = 2, groups=None) -> bass.Bass:
    if groups is None:
        groups = GROUPS
    nc = bacc.Bacc("TRN2", target_bir_lowering=False, debug=False)

    xc = nc.dram_tensor("xc", [BPC, 128, CVTOT], BF16, kind="ExternalInput")
    wm = nc.dram_tensor("wm", [128, 2 * 4 * 4 * 128], BF16, kind="ExternalInput")
    br = nc.dram_tensor("br", [128, 2], F32, kind="ExternalInput")
    out = nc.dram_tensor("out", [128, 2 * BPC], F32, kind="ExternalOutput")

    Id = mybir.ActivationFunctionType.Identity
    Tanh = mybir.ActivationFunctionType.Tanh
    MAX = mybir.AluOpType.max
    MIN = mybir.AluOpType.min

    with ExitStack() as ctx:
        tc = ctx.enter_context(tile.TileContext(nc))
        consts = ctx.enter_context(tc.tile_pool(name="consts", bufs=1))
        canvp = ctx.enter_context(tc.tile_pool(name="canv", bufs=3))
        psump = ctx.enter_context(tc.tile_pool(name="ps", bufs=2, space="PSUM"))
        evacp = ctx.enter_context(tc.tile_pool(name="ev", bufs=8))
        mpool = ctx.enter_context(tc.tile_pool(name="mt", bufs=3))
        accp = ctx.enter_context(tc.tile_pool(name="acc", bufs=4))

        w_sb = consts.tile([128, 2 * 4 * 4 * 128], BF16, tag="w")
        nc.sync.dma_start(w_sb[:], wm[:, :])
        b_sb = consts.tile([128, 2], F32, tag="b")
        nc.sync.dma_start(b_sb[:], br[:, :])
        s_all = consts.tile([128, 2 * BPC], F32, tag="sums")
        nc.vector.memset(s_all[:], 0.0)
        o_sb = consts.tile([128, 2 * BPC], F32, tag="out")

        for img in range(n_imgs):
            canv = canvp.tile([128, CVTOT], BF16, tag="canv")
            nc.sync.dma_start(canv[:], xc[img])
            cv = canv[:].rearrange("p (r c) -> p r c", c=WP)
            for half in range(n_halves):
                acc = accp.tile([128, len(groups)], F32, tag="acc")
                for g, chunks in enumerate(groups):
                    nch = len(chunks)
                    evs = []
                    for p in range(4):
                        ph, pw = p >> 1, p & 1
                        ps = psump.tile([128, 4, 512], F32, tag="ps")
                        for ci, ch in enumerate(chunks):
                            r0 = CHUNK_R0[ch]
                            nr = CHUNK_ROWS[ch]
                            for t in range(4):
                                a, bb = t >> 1, t & 1
                                di, _kh = _tap(ph, a)
                                dj, _kw = _tap(pw, bb)
                                col = _wcol(half, p, t)
                                nc.tensor.matmul(
                                    ps[:, ci, : nr * 64],
                                    w_sb[:, col : col + 128],
                                    cv[:, r0 + di : r0 + di + nr, 1 + dj : 65 + dj],
                                    start=(t == 0),
                                    stop=(t == 3),
                                )
                        # Evacuate this phase's group (PSUM fp32 -> SBUF
                        # bf16) on ScalarE with the bias add fused in.
                        ev = evacp.tile([128, 4, 512], BF16, tag="ev")
                        nc.scalar.activation(
                            ev[:, :nch, :],
                            ps[:, :nch, :],
                            Id,
                            bias=b_sb[:, half : half + 1],
                        )
                        evs.append(ev)

                    # max over the 4 phases (= the 2x2 maxpool), then
                    # hardtanh clip, then sum -> one fp32 partial per group.
                    nf = nch * 512
                    m01 = mpool.tile([128, 4 * 512], BF16, tag="m01")
                    nc.vector.tensor_tensor(
                        m01[:, :nf],
                        evs[0][:].rearrange("p a b -> p (a b)")[:, :nf],
                        evs[1][:].rearrange("p a b -> p (a b)")[:, :nf],
                        MAX,
                    )
                    m23 = mpool.tile([128, 4 * 512], BF16, tag="m23")
                    nc.vector.tensor_tensor(
                        m23[:, :nf],
                        evs[2][:].rearrange("p a b -> p (a b)")[:, :nf],
                        evs[3][:].rearrange("p a b -> p (a b)")[:, :nf],
                        MAX,
                    )
                    nc.vector.tensor_tensor(m01[:, :nf], m01[:, :nf], m23[:, :nf], MAX)
                    nc.vector.tensor_scalar(
                        out=m01[:, :nf],
                        in0=m01[:, :nf],
                        scalar1=-1.0,
                        scalar2=1.0,
                        op0=MAX,
                        op1=MIN,
                    )
                    nc.vector.tensor_reduce(
                        acc[:, g : g + 1],
                        m01[:, :nf],
                        axis=mybir.AxisListType.X,
                        op=mybir.AluOpType.add,
                    )
                idx = img * 2 + half
                nc.vector.reduce_sum(
                    s_all[:, idx : idx + 1],
                    acc[:, : len(groups)],
                    axis=mybir.AxisListType.X,
                )

        nc.scalar.activation(o_sb[:], s_all[:], Tanh, scale=1.0 / 4096.0)
        nc.sync.dma_start(out[:, :], o_sb[:])

    nc.finalize()
    return nc


_CACHE: dict = {}


def _get_nc() -> bass.Bass:
    if "nc" not in _CACHE:
        _CACHE["nc"] = build_nc()
    return _CACHE["nc"]


def make_in_maps(x: np.ndarray, weight: np.ndarray, bias: np.ndarray):
    x = np.asarray(x, dtype=np.float32)
    weight = np.asarray(weight, dtype=np.float32)
    bias = np.asarray(bias, dtype=np.float32)

    canv = np.zeros((B, 128, CVTOT), dtype=ml_dtypes.bfloat16)
    view = canv.reshape(B, 128, NROW, WP)
    view[:, :, 1:65, 1:65] = x  # cast fp32 -> bf16

    wmv = np.zeros((128, 2 * 4 * 4 * 128), dtype=ml_dtypes.bfloat16)
    for half in range(2):
        for p in range(4):
            ph, pw = p >> 1, p & 1
            for t in range(4):
                a, bb = t >> 1, t & 1
                _di, kh = _tap(ph, a)
                _dj, kw = _tap(pw, bb)
                col = _wcol(half, p, t)
                wmv[:, col : col + 128] = weight[
                    :, half * 128 : (half + 1) * 128, kh, kw
                ]

    brv = np.ascontiguousarray(bias.reshape(2, 128).T, dtype=np.float32)

    return [
        {"xc": canv[c * BPC : (c + 1) * BPC], "wm": wmv, "br": brv}
        for c in range(NCORES)
    ]


def assemble_output(results: list) -> np.ndarray:
    outs = []
    for c in range(NCORES):
        o = np.asarray(results[c]["out"])  # [128, 2*BPC]
        o = o.reshape(128, BPC, 2).transpose(1, 2, 0).reshape(BPC, COUT)
        outs.append(o)
    return np.concatenate(outs, 0).reshape(B, COUT, 1, 1).astype(np.float32)


def kernel(x: np.ndarray, weight: np.ndarray, bias: np.ndarray) -> np.ndarray:
    nc = _get_nc()
    in_maps = make_in_maps(x, weight, bias)
    res = run_bass_kernel_spmd(nc, in_maps, core_ids=list(range(NCORES)))
    return assemble_output(res.results)
